# revision 7
# baseline (speedup 1.0000x reference)
"""Trainium2 Bass kernel for the MCAT gated-attention MIL pooling model.

Math (reference after dead-code elimination + algebraic folding):
  The per-instance cross-attention softmax is over a length-1 axis -> attn_w == 1,
  so fused = v = h @ Wv + bv with h = relu(x_path @ W1 + b1).  The x_cell / wq /
  wk branch is dead.

  Key folding: f( = v) is LINEAR in h, so
    - gating:  f @ Wa = h @ (Wv Wa) + (bv Wa)   -> composed weights on the host
    - pooling: sum_n w_n f_n = (sum_n w_n h_n) @ Wv + bv * sum_n w_n
  The device therefore never materializes f at all:
      h   = relu(x @ W1 + b1)                  (N, 256)
      a   = tanh(h @ Wa' + ba')                Wa' = Wv Wa,      ba' = bv Wa + ba
      t   = tanh(h @ Wb' + bb')                Wb' = 0.5 Wv Wb,  bb' = 0.5 (bv Wb + bb)
      A   = (a * (1 + t)) @ (0.5 ac)           (sigmoid(y) = 0.5 (1 + tanh(y/2)))
      w   = exp(A)          (the ac_b bias cancels in S/Z and is dropped)
      S  += w_n * h_n ;  Z += w_n              per-core partial sums
  Host: pooled = (S/Z) @ Wv + bv ; risk = relu(pooled @ c1 + b) @ c2 + b2  (fp64).

Precision: rel-err budget is 2e-2; measured host study gives 2.3e-3 with x/W1/h
and the gating weights in fp8(e4m3, power-of-2 scaled) and everything else bf16.
fp8 enables DoubleRow matmuls (2 contraction rows per PE cell) for the dominant
x@W1 (8 MMs/block instead of 16) and the gating projections (2 each instead of 4).
Scales are powers of two folded into ACT/DVE epilogues (exact).

Sharding: rows split across 8 cores (6250 each); cores return per-block partial
sums S (128,2,NB) and Z (1,NB); host reduces in fp64 + tiny classifier.
"""

import sys
from contextlib import ExitStack

import numpy as np

try:
    import concourse  # noqa: F401
except ImportError:  # pragma: no cover - fresh grading env
    sys.path.insert(0, "/opt/trn_rl_repo")

import ml_dtypes

import concourse.bass as bass
import concourse.tile as tile
from concourse import bacc, mybir
from concourse.bass_utils import run_bass_kernel_spmd

N_CORES = 8
N = 50000
NPC = N // N_CORES  # 6250 rows per core
D_IN = 1024
D_HID = 256
NB = 512  # instances per block (one PSUM bank of fp32)
USE_DR = True  # DoubleRow fp8 matmuls (2 contraction rows/cell)

F32 = mybir.dt.float32
BF16 = mybir.dt.bfloat16
FP8 = mybir.dt.float8e4
AF = mybir.ActivationFunctionType
ALU = mybir.AluOpType
DR = mybir.MatmulPerfMode.DoubleRow

NP_FP8 = ml_dtypes.float8_e4m3
NP_BF16 = ml_dtypes.bfloat16

# power-of-2 quantization scales (folded back out in on-chip epilogues)
S_X = 16.0
S_W1 = 1024.0
S_H = 32.0
S_WAB = 4096.0
SC_H = S_H / (S_X * S_W1)  # psum -> h units
SC_AT = 1.0 / (S_H * S_WAB)  # gating psum -> pre-activation units


def _build_tile_kernel(ctx: ExitStack, tc: tile.TileContext, t, npc, nblocks, zero_bias):
    nc = tc.nc

    singles = ctx.enter_context(tc.tile_pool(name="singles", bufs=1))
    xpool = ctx.enter_context(tc.tile_pool(name="xp", bufs=5))
    actp = ctx.enter_context(tc.tile_pool(name="actp", bufs=3))
    psum = ctx.enter_context(tc.tile_pool(name="psum", bufs=2, space=bass.MemorySpace.PSUM))

    # Block-0 x DMA first in program order, split per 256-feature chunk so the
    # first h-matmul can start as soon as chunk 0 lands (subtile deps).
    x_tiles0 = xpool.tile([128, 4, 2, NB], FP8, tag="x")
    for c in range(4):
        nc.sync.dma_start(
            out=x_tiles0[:, c],
            in_=t["xt"][:, c * 2 * NB : (c + 1) * 2 * NB].rearrange("p (i j) -> p i j", j=NB),
        )

    # ---- persistent weights / biases in SBUF --------------------------------
    # w1 split per chunk for the same early-start reason.
    w1_sb = singles.tile([128, 4, 2, 2, 128], FP8, name="w1_sb")
    for c in range(4):
        nc.scalar.dma_start(out=w1_sb[:, c],
                            in_=t["w1q"][:, c * 512 : (c + 1) * 512].rearrange("p (i m j) -> p i m j", m=2, j=128))
    wa_sb = singles.tile([128, 2, 2, 128], FP8, name="wa_sb")
    nc.scalar.dma_start(out=wa_sb, in_=t["waq"].rearrange("p (i m j) -> p i m j", m=2, j=128))
    wb_sb = singles.tile([128, 2, 2, 128], FP8, name="wb_sb")
    nc.scalar.dma_start(out=wb_sb, in_=t["wbq"].rearrange("p (i m j) -> p i m j", m=2, j=128))
    ac_sb = singles.tile([128, 2, 1], BF16, name="ac_sb")
    nc.scalar.dma_start(out=ac_sb, in_=t["ach"].rearrange("p (k o) -> p k o", o=1))

    if not zero_bias:
        b1_sb = singles.tile([128, 2], F32, name="b1_sb")
        nc.scalar.dma_start(out=b1_sb, in_=t["b1s"].rearrange("(m p) -> p m", p=128))
        ba_sb = singles.tile([128, 2], F32, name="ba_sb")
        nc.scalar.dma_start(out=ba_sb, in_=t["bas"].rearrange("(m p) -> p m", p=128))
        bb_sb = singles.tile([128, 2], F32, name="bb_sb")
        nc.scalar.dma_start(out=bb_sb, in_=t["bbs"].rearrange("(m p) -> p m", p=128))

    s_parts = singles.tile([128, 2, nblocks], F32)
    z_parts = singles.tile([1, nblocks], F32)

    h_tiles = {}
    g_tiles = {}

    def h_phase(b):
        nb = min(NB, npc - b * NB)
        if b == 0:
            x_tile = x_tiles0
        else:
            x_tile = xpool.tile([128, 4, 2, NB], FP8, tag="x")
            nc.sync.dma_start(
                out=x_tile,
                in_=t["xt"][:, b * 8 * NB : (b + 1) * 8 * NB].rearrange("p (c i j) -> p c i j", i=2, j=NB),
            )

        # h^T = relu(W1^T x^T + b1), stored as fp8 (scaled by S_H).
        # Per-m psum banks with bufs=1: relu(m) drains while the other m's
        # matmuls run, so the next block's matmuls never wait.
        h_sb = actp.tile([128, 2, NB], FP8, tag="h", bufs=4)
        for m in range(2):
            ph = psum.tile([128, NB], F32, tag=f"ph{m}", bufs=1)
            if USE_DR:
                for c in range(4):
                    nc.tensor.matmul(ph[:, :nb], w1_sb[:, c, :, m, :], x_tile[:, c, :, :nb],
                                     perf_mode=DR, start=(c == 0), stop=(c == 3))
            else:
                for c in range(4):
                    for i in range(2):
                        nc.tensor.matmul(ph[:, :nb], w1_sb[:, c, i, m, :], x_tile[:, c, i, :nb],
                                         start=(c == 0 and i == 0), stop=(c == 3 and i == 1))
            if zero_bias:
                nc.vector.tensor_scalar(out=h_sb[:, m, :nb], in0=ph[:, :nb], scalar1=SC_H,
                                        scalar2=0.0, op0=ALU.mult, op1=ALU.max)
            else:
                nc.scalar.activation(out=h_sb[:, m, :nb], in_=ph[:, :nb], func=AF.Relu,
                                     bias=b1_sb[:, m : m + 1], scale=SC_H)
        h_tiles[b] = h_sb

    def gate_a(b):
        """a/t projections + tanh + u = a*t."""
        nb = min(NB, npc - b * NB)
        h_sb = h_tiles[b]

        # a = tanh(h Wa' + ba');  t = tanh(h Wb' + bb')  (0.5s folded host-side)
        # One 4-bank psum tile -> a single merged tanh op over [128, 4, nb].
        pat = psum.tile([128, 4, NB], F32, tag="pat", bufs=1)
        at_sb = actp.tile([128, 4, NB], BF16, tag="at")
        for m in range(2):
            if USE_DR:
                nc.tensor.matmul(pat[:, 0 + m, :nb], wa_sb[:, :, m, :], h_sb[:, :, :nb], perf_mode=DR)
                nc.tensor.matmul(pat[:, 2 + m, :nb], wb_sb[:, :, m, :], h_sb[:, :, :nb], perf_mode=DR)
            else:
                for i in range(2):
                    nc.tensor.matmul(pat[:, 0 + m, :nb], wa_sb[:, i, m, :], h_sb[:, i, :nb],
                                     start=(i == 0), stop=(i == 1))
                for i in range(2):
                    nc.tensor.matmul(pat[:, 2 + m, :nb], wb_sb[:, i, m, :], h_sb[:, i, :nb],
                                     start=(i == 0), stop=(i == 1))
        if zero_bias:
            nc.scalar.activation(out=at_sb[:, :, :nb], in_=pat[:, :, :nb], func=AF.Tanh, scale=SC_AT)
        else:
            for m in range(2):
                nc.scalar.activation(out=at_sb[:, 0 + m, :nb], in_=pat[:, 0 + m, :nb], func=AF.Tanh,
                                     bias=ba_sb[:, m : m + 1], scale=SC_AT)
                nc.scalar.activation(out=at_sb[:, 2 + m, :nb], in_=pat[:, 2 + m, :nb], func=AF.Tanh,
                                     bias=bb_sb[:, m : m + 1], scale=SC_AT)

        # u = a * t  (a*(1+t) = a + a*t is folded into two A-projections)
        u_sb = actp.tile([128, 2, NB], BF16, tag="u")
        nc.vector.tensor_tensor(out=u_sb[:, :, :nb], in0=at_sb[:, 0:2, :nb],
                                in1=at_sb[:, 2:4, :nb], op=ALU.mult)
        g_tiles[b] = (at_sb, u_sb)

    def gate_b(b):
        """A projection, softmax weight, weighted pooling partials."""
        nb = min(NB, npc - b * NB)
        h_sb = h_tiles.pop(b)
        at_sb, u_sb = g_tiles.pop(b)

        # A = (a + a*t) @ (0.5 ac) -> (1, nb);  w = exp(A); Z += sum(w)
        pA = psum.tile([1, NB], F32, tag="pA", bufs=1)
        for k in range(2):
            nc.tensor.matmul(pA[:, :nb], ac_sb[:, k, :], at_sb[:, k, :nb], start=(k == 0), stop=False)
        for k in range(2):
            nc.tensor.matmul(pA[:, :nb], ac_sb[:, k, :], u_sb[:, k, :nb], start=False, stop=(k == 1))
        w_sb = actp.tile([1, NB], BF16, tag="w")
        nc.scalar.activation(out=w_sb[:, :nb], in_=pA[:, :nb], func=AF.Exp, scale=1.0,
                             accum_out=z_parts[:, b : b + 1])

        # broadcast w to all partitions (GpSimd), then S[:,m,b] += rowsum(h/S_H * w)
        wb_bc = actp.tile([128, NB], BF16, tag="wb")
        nc.gpsimd.partition_broadcast(wb_bc[:, :nb], w_sb[:, :nb])
        for m in range(2):
            wf = actp.tile([128, NB], BF16, tag="wf")
            nc.vector.scalar_tensor_tensor(out=wf[:, :nb], in0=h_sb[:, m, :nb], scalar=1.0 / S_H,
                                           in1=wb_bc[:, :nb], op0=ALU.mult, op1=ALU.mult,
                                           accum_out=s_parts[:, m, b : b + 1])

    # Software pipeline: gate_a runs one block late, gate_b two blocks late,
    # so no engine FIFO ever stalls on the cross-engine chain
    # (relu -> a/t MMs -> tanh -> g -> A MM -> exp -> bcast -> weighted sum)
    # and the PE stays continuously busy (HAM stays warm).
    for b in range(nblocks):
        h_phase(b)
        if b >= 1:
            gate_a(b - 1)
        if b >= 2:
            gate_b(b - 2)
    gate_a(nblocks - 1)
    gate_b(nblocks - 2)
    gate_b(nblocks - 1)

    nc.sync.dma_start(out=t["s_out"], in_=s_parts)
    nc.sync.dma_start(out=t["z_out"], in_=z_parts)


def build_program(npc: int = NPC, zero_bias: bool = True, enable_asserts: bool = False):
    nblocks = (npc + NB - 1) // NB
    nc = bacc.Bacc("TRN2", target_bir_lowering=False, debug=False, enable_asserts=enable_asserts)

    t = {}
    t["xt"] = nc.dram_tensor("xt", [128, nblocks * 8 * NB], FP8, kind="ExternalInput").ap()
    t["w1q"] = nc.dram_tensor("w1q", [128, 2048], FP8, kind="ExternalInput").ap()
    t["waq"] = nc.dram_tensor("waq", [128, 512], FP8, kind="ExternalInput").ap()
    t["wbq"] = nc.dram_tensor("wbq", [128, 512], FP8, kind="ExternalInput").ap()
    t["ach"] = nc.dram_tensor("ach", [128, 2], BF16, kind="ExternalInput").ap()
    if not zero_bias:
        for nm in ("b1s", "bas", "bbs"):
            t[nm] = nc.dram_tensor(nm, [D_HID], F32, kind="ExternalInput").ap()
    t["s_out"] = nc.dram_tensor("s_out", [128, 2, nblocks], F32, kind="ExternalOutput").ap()
    t["z_out"] = nc.dram_tensor("z_out", [1, nblocks], F32, kind="ExternalOutput").ap()

    with tile.TileContext(nc) as tc, ExitStack() as ctx:
        _build_tile_kernel(ctx, tc, t, npc, nblocks, zero_bias)
    nc.compile()
    return nc


def _q8(a: np.ndarray, scale: float) -> np.ndarray:
    return np.ascontiguousarray((np.asarray(a, np.float32) * scale).astype(NP_FP8))


def make_weight_map(inputs, zero_bias=None):
    W1 = np.asarray(inputs["wsi_w"], np.float64)
    Wv = np.asarray(inputs["wv_w"], np.float64)
    Wa = np.asarray(inputs["aa_w"], np.float64)
    Wb = np.asarray(inputs["ab_w"], np.float64)
    ac = np.asarray(inputs["ac_w"], np.float64)
    bv = np.asarray(inputs["wv_b"], np.float64)
    b1 = np.asarray(inputs["wsi_b"], np.float64)
    ba = np.asarray(bv @ Wa + np.asarray(inputs["aa_b"], np.float64))
    bb = np.asarray(0.5 * (bv @ Wb + np.asarray(inputs["ab_b"], np.float64)))

    # composed gating weights (f folded away); 0.5 of the tanh-sigmoid in Wb'
    Wa_c = Wv @ Wa
    Wb_c = 0.5 * (Wv @ Wb)

    # device layouts
    w1q = _q8(W1, S_W1).reshape(4, 2, 128, 2, 128).transpose(2, 0, 1, 3, 4).reshape(128, 2048)
    waq = _q8(Wa_c, S_WAB).reshape(2, 128, 2, 128).transpose(1, 0, 2, 3).reshape(128, 512)
    wbq = _q8(Wb_c, S_WAB).reshape(2, 128, 2, 128).transpose(1, 0, 2, 3).reshape(128, 512)
    ach = np.ascontiguousarray(
        (0.5 * ac).astype(NP_BF16).reshape(2, 128, 1).transpose(1, 0, 2).reshape(128, 2)
    )
    m = {"w1q": np.ascontiguousarray(w1q), "waq": np.ascontiguousarray(waq),
         "wbq": np.ascontiguousarray(wbq), "ach": ach}
    zb = not (np.any(b1) or np.any(ba) or np.any(bb))
    if not zb:
        m["b1s"] = (np.asarray(b1, np.float32) * S_H).astype(np.float32)
        m["bas"] = np.asarray(ba, np.float32)
        m["bbs"] = np.asarray(bb, np.float32)
    m["_zero_bias"] = zb
    return m


def make_in_maps(x_path, weights, npc: int = NPC, n_cores: int = N_CORES):
    x = np.asarray(x_path[0], np.float32)  # (N, 1024)
    nblocks = (npc + NB - 1) // NB
    npad = nblocks * NB
    w = {k: v for k, v in weights.items() if not k.startswith("_")}
    in_maps = []
    for c in range(n_cores):
        xc = np.zeros((npad, D_IN), np.float32)
        xc[:npc] = x[c * npc : (c + 1) * npc]
        xq = (xc * S_X).astype(NP_FP8)
        # [inst, feat] -> [p, (b c i j)] with feat = c*256 + i*128 + p
        packed = np.ascontiguousarray(
            xq.reshape(nblocks, NB, 4, 2, 128).transpose(4, 0, 2, 3, 1).reshape(128, nblocks * 8 * NB)
        )
        in_maps.append({"xt": packed, **w})
    return in_maps


def finalize(results, c1_w, c1_b, c2_w, c2_b, wv_w, wv_b):
    """Host-side reduction of per-core partials, Wv application + classifier."""
    S = np.zeros((128, 2), np.float64)
    Z = 0.0
    for r in results:
        S += np.asarray(r["s_out"], np.float64).sum(axis=-1)
        Z += float(np.asarray(r["z_out"], np.float64).sum())
    s_vec = S.T.reshape(256)  # feature = m*128 + p
    pooled = (s_vec / Z) @ np.asarray(wv_w, np.float64) + np.asarray(wv_b, np.float64)
    risk = (
        np.maximum(pooled @ np.asarray(c1_w, np.float64) + np.asarray(c1_b, np.float64), 0.0)
        @ np.asarray(c2_w, np.float64)
        + np.asarray(c2_b, np.float64)
    )
    return risk[None, :].astype(np.float32)


_CACHED = {}


def kernel(**inputs) -> np.ndarray:
    weights = make_weight_map(inputs)
    zb = weights["_zero_bias"]
    if zb not in _CACHED:
        _CACHED[zb] = build_program(zero_bias=zb)
    nc = _CACHED[zb]

    in_maps = make_in_maps(np.asarray(inputs["x_path"]), weights)
    res = run_bass_kernel_spmd(nc, in_maps, list(range(N_CORES)))
    return finalize(
        res.results,
        inputs["c1_w"], inputs["c1_b"], inputs["c2_w"], inputs["c2_b"],
        inputs["wv_w"], inputs["wv_b"],
    )


# revision 19
# speedup vs baseline: 1.0175x; 1.0175x over previous
"""Trainium2 Bass kernel for the MCAT gated-attention MIL pooling model.

Math (reference after dead-code elimination + algebraic folding):
  The per-instance cross-attention softmax is over a length-1 axis -> attn_w == 1,
  so fused = v = h @ Wv + bv with h = relu(x_path @ W1 + b1).  The x_cell / wq /
  wk branch is dead.

  Key folding: f( = v) is LINEAR in h, so
    - gating:  f @ Wa = h @ (Wv Wa) + (bv Wa)   -> composed weights on the host
    - pooling: sum_n w_n f_n = (sum_n w_n h_n) @ Wv + bv * sum_n w_n
  The device therefore never materializes f at all:
      h   = relu(x @ W1 + b1)                  (N, 256)
      a   = tanh(h @ Wa' + ba')                Wa' = Wv Wa,      ba' = bv Wa + ba
      t   = tanh(h @ Wb' + bb')                Wb' = 0.5 Wv Wb,  bb' = 0.5 (bv Wb + bb)
      A   = (a * (1 + t)) @ (0.5 ac)           (sigmoid(y) = 0.5 (1 + tanh(y/2)))
      w   = exp(A)          (the ac_b bias cancels in S/Z and is dropped)
      S  += w_n * h_n ;  Z += w_n              per-core partial sums
  Host: pooled = (S/Z) @ Wv + bv ; risk = relu(pooled @ c1 + b) @ c2 + b2  (fp64).

Precision: rel-err budget is 2e-2; measured host study gives 2.3e-3 with x/W1/h
and the gating weights in fp8(e4m3, power-of-2 scaled) and everything else bf16.
fp8 enables DoubleRow matmuls (2 contraction rows per PE cell) for the dominant
x@W1 (8 MMs/block instead of 16) and the gating projections (2 each instead of 4).
Scales are powers of two folded into ACT/DVE epilogues (exact).

Sharding: rows split across 8 cores (6250 each); cores return per-block partial
sums S (128,2,NB) and Z (1,NB); host reduces in fp64 + tiny classifier.
"""

import sys
from contextlib import ExitStack

import numpy as np

try:
    import concourse  # noqa: F401
except ImportError:  # pragma: no cover - fresh grading env
    sys.path.insert(0, "/opt/trn_rl_repo")

import ml_dtypes

import concourse.bass as bass
import concourse.tile as tile
from concourse import bacc, mybir
from concourse.bass_utils import run_bass_kernel_spmd

N_CORES = 8
N = 50000
NPC = N // N_CORES  # 6250 rows per core
D_IN = 1024
D_HID = 256
NB = 512  # instances per block (one PSUM bank of fp32)
USE_DR = True  # DoubleRow fp8 matmuls (2 contraction rows/cell)

F32 = mybir.dt.float32
BF16 = mybir.dt.bfloat16
FP8 = mybir.dt.float8e4
AF = mybir.ActivationFunctionType
ALU = mybir.AluOpType
DR = mybir.MatmulPerfMode.DoubleRow

NP_FP8 = ml_dtypes.float8_e4m3
NP_BF16 = ml_dtypes.bfloat16

# power-of-2 quantization scales (folded back out in on-chip epilogues)
S_X = 16.0
S_W1 = 1024.0
S_H = 32.0
S_WAB = 4096.0
SC_H = S_H / (S_X * S_W1)  # psum -> h units
SC_AT = 1.0 / (S_H * S_WAB)  # gating psum -> pre-activation units


def _build_tile_kernel(ctx: ExitStack, tc: tile.TileContext, t, npc, nblocks, zero_bias):
    nc = tc.nc

    singles = ctx.enter_context(tc.tile_pool(name="singles", bufs=1))
    xpool = ctx.enter_context(tc.tile_pool(name="xp", bufs=6))
    actp = ctx.enter_context(tc.tile_pool(name="actp", bufs=3))
    psum = ctx.enter_context(tc.tile_pool(name="psum", bufs=2, space=bass.MemorySpace.PSUM))

    # Block-0 x DMA first in program order, split per 256-feature chunk so the
    # first h-matmul can start as soon as chunk 0 lands (subtile deps).
    x_tiles0 = xpool.tile([128, 4, 2, NB], FP8, tag="x")
    for c in range(4):
        nc.sync.dma_start(
            out=x_tiles0[:, c],
            in_=t["xt"][:, c * 2 * NB : (c + 1) * 2 * NB].rearrange("p (i j) -> p i j", j=NB),
        )

    # ---- persistent weights / biases in SBUF --------------------------------
    # w1 split per chunk for the same early-start reason.
    w1_sb = singles.tile([128, 4, 2, 2, 128], FP8, name="w1_sb")
    for c in range(4):
        nc.scalar.dma_start(out=w1_sb[:, c],
                            in_=t["w1q"][:, c * 512 : (c + 1) * 512].rearrange("p (i m j) -> p i m j", m=2, j=128))
    wa_sb = singles.tile([128, 2, 2, 128], FP8, name="wa_sb")
    nc.scalar.dma_start(out=wa_sb, in_=t["waq"].rearrange("p (i m j) -> p i m j", m=2, j=128))
    wb_sb = singles.tile([128, 2, 2, 128], FP8, name="wb_sb")
    nc.scalar.dma_start(out=wb_sb, in_=t["wbq"].rearrange("p (i m j) -> p i m j", m=2, j=128))
    ac_sb = singles.tile([128, 2, 1], BF16, name="ac_sb")
    nc.scalar.dma_start(out=ac_sb, in_=t["ach"].rearrange("p (k o) -> p k o", o=1))

    if not zero_bias:
        b1_sb = singles.tile([128, 2], F32, name="b1_sb")
        nc.scalar.dma_start(out=b1_sb, in_=t["b1s"].rearrange("(m p) -> p m", p=128))
        ba_sb = singles.tile([128, 2], F32, name="ba_sb")
        nc.scalar.dma_start(out=ba_sb, in_=t["bas"].rearrange("(m p) -> p m", p=128))
        bb_sb = singles.tile([128, 2], F32, name="bb_sb")
        nc.scalar.dma_start(out=bb_sb, in_=t["bbs"].rearrange("(m p) -> p m", p=128))

    s_parts = singles.tile([128, 2, nblocks], F32)
    z_parts = singles.tile([1, nblocks], F32)

    # PE warmup: ~3.4us of dummy matmuls on zeros while the first x/w DMAs are
    # in flight, so the HAM clock-gate is already at full rate (2.4 GHz) when
    # the real matmuls start.
    zer_sb = singles.tile([128, NB], BF16, name="zer_sb")
    nc.vector.memset(zer_sb, 0.0)
    pwarm = psum.tile([1, NB], F32, tag="pA", bufs=1)
    for _ in range(8):
        nc.tensor.matmul(pwarm, zer_sb[:, 0:1], zer_sb, start=True, stop=True)

    h_tiles = {}
    g_tiles = {}

    def h_phase(b):
        nb = min(NB, npc - b * NB)
        if b == 0:
            x_tile = x_tiles0
        else:
            x_tile = xpool.tile([128, 4, 2, NB], FP8, tag="x")
            nc.sync.dma_start(
                out=x_tile,
                in_=t["xt"][:, b * 8 * NB : (b + 1) * 8 * NB].rearrange("p (c i j) -> p c i j", i=2, j=NB),
            )

        # h^T = relu(W1^T x^T + b1), stored as fp8 (scaled by S_H).
        # Per-m psum banks with bufs=1: relu(m) drains while the other m's
        # matmuls run, so the next block's matmuls never wait.
        h_sb = actp.tile([128, 2, NB], FP8, tag="h", bufs=4)
        for m in range(2):
            ph = psum.tile([128, NB], F32, tag=f"ph{m}", bufs=1)
            if USE_DR:
                for c in range(4):
                    nc.tensor.matmul(ph[:, :nb], w1_sb[:, c, :, m, :], x_tile[:, c, :, :nb],
                                     perf_mode=DR, start=(c == 0), stop=(c == 3))
            else:
                for c in range(4):
                    for i in range(2):
                        nc.tensor.matmul(ph[:, :nb], w1_sb[:, c, i, m, :], x_tile[:, c, i, :nb],
                                         start=(c == 0 and i == 0), stop=(c == 3 and i == 1))
            if zero_bias:
                nc.vector.tensor_scalar(out=h_sb[:, m, :nb], in0=ph[:, :nb], scalar1=SC_H,
                                        scalar2=0.0, op0=ALU.mult, op1=ALU.max)
            else:
                nc.scalar.activation(out=h_sb[:, m, :nb], in_=ph[:, :nb], func=AF.Relu,
                                     bias=b1_sb[:, m : m + 1], scale=SC_H)
        h_tiles[b] = h_sb

    def gate_a(b):
        """a/t projections + tanh + u = a*t."""
        nb = min(NB, npc - b * NB)
        h_sb = h_tiles[b]

        # a = tanh(h Wa' + ba');  t = tanh(h Wb' + bb')  (0.5s folded host-side)
        # One 4-bank psum tile -> a single merged tanh op over [128, 4, nb].
        pat = psum.tile([128, 4, NB], F32, tag="pat", bufs=1)
        at_sb = actp.tile([128, 4, NB], BF16, tag="at")
        for m in range(2):
            if USE_DR:
                nc.tensor.matmul(pat[:, 0 + m, :nb], wa_sb[:, :, m, :], h_sb[:, :, :nb], perf_mode=DR)
                nc.tensor.matmul(pat[:, 2 + m, :nb], wb_sb[:, :, m, :], h_sb[:, :, :nb], perf_mode=DR)
            else:
                for i in range(2):
                    nc.tensor.matmul(pat[:, 0 + m, :nb], wa_sb[:, i, m, :], h_sb[:, i, :nb],
                                     start=(i == 0), stop=(i == 1))
                for i in range(2):
                    nc.tensor.matmul(pat[:, 2 + m, :nb], wb_sb[:, i, m, :], h_sb[:, i, :nb],
                                     start=(i == 0), stop=(i == 1))
        if zero_bias:
            nc.scalar.activation(out=at_sb[:, :, :nb], in_=pat[:, :, :nb], func=AF.Tanh, scale=SC_AT)
        else:
            for m in range(2):
                nc.scalar.activation(out=at_sb[:, 0 + m, :nb], in_=pat[:, 0 + m, :nb], func=AF.Tanh,
                                     bias=ba_sb[:, m : m + 1], scale=SC_AT)
                nc.scalar.activation(out=at_sb[:, 2 + m, :nb], in_=pat[:, 2 + m, :nb], func=AF.Tanh,
                                     bias=bb_sb[:, m : m + 1], scale=SC_AT)

        # u = a * t  (a*(1+t) = a + a*t is folded into two A-projections)
        u_sb = actp.tile([128, 2, NB], BF16, tag="u")
        nc.vector.tensor_tensor(out=u_sb[:, :, :nb], in0=at_sb[:, 0:2, :nb],
                                in1=at_sb[:, 2:4, :nb], op=ALU.mult)
        g_tiles[b] = (at_sb, u_sb)

    def gate_b(b):
        """A projection, softmax weight, weighted pooling partials."""
        nb = min(NB, npc - b * NB)
        h_sb = h_tiles.pop(b)
        at_sb, u_sb = g_tiles.pop(b)

        # A = (a + a*t) @ (0.5 ac) -> (1, nb);  w = exp(A); Z += sum(w)
        pA = psum.tile([1, NB], F32, tag="pA", bufs=1)
        for k in range(2):
            nc.tensor.matmul(pA[:, :nb], ac_sb[:, k, :], at_sb[:, k, :nb], start=(k == 0), stop=False)
        for k in range(2):
            nc.tensor.matmul(pA[:, :nb], ac_sb[:, k, :], u_sb[:, k, :nb], start=False, stop=(k == 1))
        w_sb = actp.tile([1, NB], BF16, tag="w")
        nc.scalar.activation(out=w_sb[:, :nb], in_=pA[:, :nb], func=AF.Exp, scale=1.0,
                             accum_out=z_parts[:, b : b + 1])

        # broadcast w to all partitions (GpSimd), then S[:,m,b] += rowsum(h/S_H * w)
        wb_bc = actp.tile([128, NB], BF16, tag="wb")
        nc.gpsimd.partition_broadcast(wb_bc[:, :nb], w_sb[:, :nb])
        for m in range(2):
            wf = actp.tile([128, NB], BF16, tag="wf")
            nc.vector.scalar_tensor_tensor(out=wf[:, :nb], in0=h_sb[:, m, :nb], scalar=1.0 / S_H,
                                           in1=wb_bc[:, :nb], op0=ALU.mult, op1=ALU.mult,
                                           accum_out=s_parts[:, m, b : b + 1])

    # Software pipeline: gate_a runs one block late, gate_b two blocks late,
    # so no engine FIFO ever stalls on the cross-engine chain
    # (relu -> a/t MMs -> tanh -> g -> A MM -> exp -> bcast -> weighted sum)
    # and the PE stays continuously busy (HAM stays warm).
    for b in range(nblocks):
        h_phase(b)
        if b >= 1:
            gate_a(b - 1)
        if b >= 2:
            gate_b(b - 2)
    gate_a(nblocks - 1)
    gate_b(nblocks - 2)
    gate_b(nblocks - 1)

    nc.sync.dma_start(out=t["s_out"], in_=s_parts)
    nc.sync.dma_start(out=t["z_out"], in_=z_parts)


def build_program(npc: int = NPC, zero_bias: bool = True, enable_asserts: bool = False):
    nblocks = (npc + NB - 1) // NB
    nc = bacc.Bacc("TRN2", target_bir_lowering=False, debug=False, enable_asserts=enable_asserts)

    t = {}
    t["xt"] = nc.dram_tensor("xt", [128, nblocks * 8 * NB], FP8, kind="ExternalInput").ap()
    t["w1q"] = nc.dram_tensor("w1q", [128, 2048], FP8, kind="ExternalInput").ap()
    t["waq"] = nc.dram_tensor("waq", [128, 512], FP8, kind="ExternalInput").ap()
    t["wbq"] = nc.dram_tensor("wbq", [128, 512], FP8, kind="ExternalInput").ap()
    t["ach"] = nc.dram_tensor("ach", [128, 2], BF16, kind="ExternalInput").ap()
    if not zero_bias:
        for nm in ("b1s", "bas", "bbs"):
            t[nm] = nc.dram_tensor(nm, [D_HID], F32, kind="ExternalInput").ap()
    t["s_out"] = nc.dram_tensor("s_out", [128, 2, nblocks], F32, kind="ExternalOutput").ap()
    t["z_out"] = nc.dram_tensor("z_out", [1, nblocks], F32, kind="ExternalOutput").ap()

    with tile.TileContext(nc) as tc, ExitStack() as ctx:
        _build_tile_kernel(ctx, tc, t, npc, nblocks, zero_bias)
    nc.compile()
    return nc


def _q8(a: np.ndarray, scale: float) -> np.ndarray:
    return np.ascontiguousarray((np.asarray(a, np.float32) * scale).astype(NP_FP8))


def make_weight_map(inputs, zero_bias=None):
    W1 = np.asarray(inputs["wsi_w"], np.float64)
    Wv = np.asarray(inputs["wv_w"], np.float64)
    Wa = np.asarray(inputs["aa_w"], np.float64)
    Wb = np.asarray(inputs["ab_w"], np.float64)
    ac = np.asarray(inputs["ac_w"], np.float64)
    bv = np.asarray(inputs["wv_b"], np.float64)
    b1 = np.asarray(inputs["wsi_b"], np.float64)
    ba = np.asarray(bv @ Wa + np.asarray(inputs["aa_b"], np.float64))
    bb = np.asarray(0.5 * (bv @ Wb + np.asarray(inputs["ab_b"], np.float64)))

    # composed gating weights (f folded away); 0.5 of the tanh-sigmoid in Wb'
    Wa_c = Wv @ Wa
    Wb_c = 0.5 * (Wv @ Wb)

    # device layouts
    w1q = _q8(W1, S_W1).reshape(4, 2, 128, 2, 128).transpose(2, 0, 1, 3, 4).reshape(128, 2048)
    waq = _q8(Wa_c, S_WAB).reshape(2, 128, 2, 128).transpose(1, 0, 2, 3).reshape(128, 512)
    wbq = _q8(Wb_c, S_WAB).reshape(2, 128, 2, 128).transpose(1, 0, 2, 3).reshape(128, 512)
    ach = np.ascontiguousarray(
        (0.5 * ac).astype(NP_BF16).reshape(2, 128, 1).transpose(1, 0, 2).reshape(128, 2)
    )
    m = {"w1q": np.ascontiguousarray(w1q), "waq": np.ascontiguousarray(waq),
         "wbq": np.ascontiguousarray(wbq), "ach": ach}
    zb = not (np.any(b1) or np.any(ba) or np.any(bb))
    if not zb:
        m["b1s"] = (np.asarray(b1, np.float32) * S_H).astype(np.float32)
        m["bas"] = np.asarray(ba, np.float32)
        m["bbs"] = np.asarray(bb, np.float32)
    m["_zero_bias"] = zb
    return m


def make_in_maps(x_path, weights, npc: int = NPC, n_cores: int = N_CORES):
    x = np.asarray(x_path[0], np.float32)  # (N, 1024)
    nblocks = (npc + NB - 1) // NB
    npad = nblocks * NB
    w = {k: v for k, v in weights.items() if not k.startswith("_")}
    in_maps = []
    for c in range(n_cores):
        xc = np.zeros((npad, D_IN), np.float32)
        xc[:npc] = x[c * npc : (c + 1) * npc]
        xq = (xc * S_X).astype(NP_FP8)
        # [inst, feat] -> [p, (b c i j)] with feat = c*256 + i*128 + p
        packed = np.ascontiguousarray(
            xq.reshape(nblocks, NB, 4, 2, 128).transpose(4, 0, 2, 3, 1).reshape(128, nblocks * 8 * NB)
        )
        in_maps.append({"xt": packed, **w})
    return in_maps


def finalize(results, c1_w, c1_b, c2_w, c2_b, wv_w, wv_b):
    """Host-side reduction of per-core partials, Wv application + classifier."""
    S = np.zeros((128, 2), np.float64)
    Z = 0.0
    for r in results:
        S += np.asarray(r["s_out"], np.float64).sum(axis=-1)
        Z += float(np.asarray(r["z_out"], np.float64).sum())
    s_vec = S.T.reshape(256)  # feature = m*128 + p
    pooled = (s_vec / Z) @ np.asarray(wv_w, np.float64) + np.asarray(wv_b, np.float64)
    risk = (
        np.maximum(pooled @ np.asarray(c1_w, np.float64) + np.asarray(c1_b, np.float64), 0.0)
        @ np.asarray(c2_w, np.float64)
        + np.asarray(c2_b, np.float64)
    )
    return risk[None, :].astype(np.float32)


_CACHED = {}


def kernel(**inputs) -> np.ndarray:
    weights = make_weight_map(inputs)
    zb = weights["_zero_bias"]
    if zb not in _CACHED:
        _CACHED[zb] = build_program(zero_bias=zb)
    nc = _CACHED[zb]

    in_maps = make_in_maps(np.asarray(inputs["x_path"]), weights)
    res = run_bass_kernel_spmd(nc, in_maps, list(range(N_CORES)))
    return finalize(
        res.results,
        inputs["c1_w"], inputs["c1_b"], inputs["c2_w"], inputs["c2_b"],
        inputs["wv_w"], inputs["wv_b"],
    )


# revision 23
# speedup vs baseline: 1.0631x; 1.0448x over previous
"""Trainium2 Bass kernel for the MCAT gated-attention MIL pooling model.

Math (reference after dead-code elimination + algebraic folding):
  The per-instance cross-attention softmax is over a length-1 axis -> attn_w == 1,
  so fused = v = h @ Wv + bv with h = relu(x_path @ W1 + b1).  The x_cell / wq /
  wk branch is dead.

  Key folding: f( = v) is LINEAR in h, so
    - gating:  f @ Wa = h @ (Wv Wa) + (bv Wa)   -> composed weights on the host
    - pooling: sum_n w_n f_n = (sum_n w_n h_n) @ Wv + bv * sum_n w_n
  The device therefore never materializes f at all:
      h   = relu(x @ W1 + b1)                  (N, 256)
      a   = tanh(h @ Wa' + ba')                Wa' = Wv Wa,      ba' = bv Wa + ba
      t   = tanh(h @ Wb' + bb')                Wb' = 0.5 Wv Wb,  bb' = 0.5 (bv Wb + bb)
      A   = (a * (1 + t)) @ (0.5 ac)           (sigmoid(y) = 0.5 (1 + tanh(y/2)))
      w   = exp(A)          (the ac_b bias cancels in S/Z and is dropped)
      S  += w_n * h_n ;  Z += w_n              per-core partial sums
  Host: pooled = (S/Z) @ Wv + bv ; risk = relu(pooled @ c1 + b) @ c2 + b2  (fp64).

Precision: rel-err budget is 2e-2; measured host study gives 2.3e-3 with x/W1/h
and the gating weights in fp8(e4m3, power-of-2 scaled) and everything else bf16.
fp8 enables DoubleRow matmuls (2 contraction rows per PE cell) for the dominant
x@W1 (8 MMs/block instead of 16) and the gating projections (2 each instead of 4).
Scales are powers of two folded into ACT/DVE epilogues (exact).

Sharding: rows split across 8 cores (6250 each); cores return per-block partial
sums S (128,2,NB) and Z (1,NB); host reduces in fp64 + tiny classifier.
"""

import sys
from contextlib import ExitStack

import numpy as np

try:
    import concourse  # noqa: F401
except ImportError:  # pragma: no cover - fresh grading env
    sys.path.insert(0, "/opt/trn_rl_repo")

import ml_dtypes

import concourse.bass as bass
import concourse.tile as tile
from concourse import bacc, mybir
from concourse.bass_utils import run_bass_kernel_spmd

N_CORES = 8
N = 50000
NPC = N // N_CORES  # 6250 rows per core
D_IN = 1024
D_HID = 256
NB = 512  # instances per block (one PSUM bank of fp32)
USE_DR = True  # DoubleRow fp8 matmuls (2 contraction rows/cell)

F32 = mybir.dt.float32
BF16 = mybir.dt.bfloat16
FP8 = mybir.dt.float8e4
AF = mybir.ActivationFunctionType
ALU = mybir.AluOpType
DR = mybir.MatmulPerfMode.DoubleRow

NP_FP8 = ml_dtypes.float8_e4m3
NP_BF16 = ml_dtypes.bfloat16

# power-of-2 quantization scales (folded back out in on-chip epilogues)
S_X = 16.0
S_W1 = 1024.0
S_H = 32.0
S_WAB = 4096.0
SC_H = S_H / (S_X * S_W1)  # psum -> h units
SC_AT = 1.0 / (S_H * S_WAB)  # gating psum -> pre-activation units


def _build_tile_kernel(ctx: ExitStack, tc: tile.TileContext, t, npc, nblocks, zero_bias):
    nc = tc.nc

    singles = ctx.enter_context(tc.tile_pool(name="singles", bufs=1))
    xpool = ctx.enter_context(tc.tile_pool(name="xp", bufs=6))
    actp = ctx.enter_context(tc.tile_pool(name="actp", bufs=3))
    psum = ctx.enter_context(tc.tile_pool(name="psum", bufs=2, space=bass.MemorySpace.PSUM))

    # Block-0 x DMA first in program order, split per 256-feature chunk so the
    # first h-matmul can start as soon as chunk 0 lands (subtile deps).
    x_tiles0 = xpool.tile([128, 4, 2, NB], FP8, tag="x")
    for c in range(4):
        nc.sync.dma_start(
            out=x_tiles0[:, c],
            in_=t["xt"][:, c * 2 * NB : (c + 1) * 2 * NB].rearrange("p (i j) -> p i j", j=NB),
        )

    # ---- persistent weights / biases in SBUF --------------------------------
    # w1 split per chunk for the same early-start reason.
    w1_sb = singles.tile([128, 4, 2, 2, 128], FP8, name="w1_sb")
    for c in range(4):
        nc.scalar.dma_start(out=w1_sb[:, c],
                            in_=t["w1q"][:, c * 512 : (c + 1) * 512].rearrange("p (i m j) -> p i m j", m=2, j=128))
    wa_sb = singles.tile([128, 2, 2, 128], FP8, name="wa_sb")
    nc.scalar.dma_start(out=wa_sb, in_=t["waq"].rearrange("p (i m j) -> p i m j", m=2, j=128))
    wb_sb = singles.tile([128, 2, 2, 128], FP8, name="wb_sb")
    nc.scalar.dma_start(out=wb_sb, in_=t["wbq"].rearrange("p (i m j) -> p i m j", m=2, j=128))
    ac_sb = singles.tile([128, 2, 1], BF16, name="ac_sb")
    nc.scalar.dma_start(out=ac_sb, in_=t["ach"].rearrange("p (k o) -> p k o", o=1))

    if not zero_bias:
        b1_sb = singles.tile([128, 2], F32, name="b1_sb")
        nc.scalar.dma_start(out=b1_sb, in_=t["b1s"].rearrange("(m p) -> p m", p=128))
        ba_sb = singles.tile([128, 2], F32, name="ba_sb")
        nc.scalar.dma_start(out=ba_sb, in_=t["bas"].rearrange("(m p) -> p m", p=128))
        bb_sb = singles.tile([128, 2], F32, name="bb_sb")
        nc.scalar.dma_start(out=bb_sb, in_=t["bbs"].rearrange("(m p) -> p m", p=128))

    s_parts = singles.tile([128, 2, nblocks], F32)
    z_parts = singles.tile([1, nblocks], F32)

    # PE warmup: ~3.4us of dummy matmuls on zeros while the first x/w DMAs are
    # in flight, so the HAM clock-gate is already at full rate (2.4 GHz) when
    # the real matmuls start.
    zer_sb = singles.tile([128, NB], BF16, name="zer_sb")
    nc.vector.memset(zer_sb, 0.0)
    pwarm = psum.tile([1, NB], F32, tag="pA", bufs=1)
    for _ in range(8):
        nc.tensor.matmul(pwarm, zer_sb[:, 0:1], zer_sb, start=True, stop=True)

    h_tiles = {}
    g_tiles = {}

    def h_phase(b):
        nb = min(NB, npc - b * NB)
        if b == 0:
            x_tile = x_tiles0
        else:
            x_tile = xpool.tile([128, 4, 2, NB], FP8, tag="x")
            nc.sync.dma_start(
                out=x_tile,
                in_=t["xt"][:, b * 8 * NB : (b + 1) * 8 * NB].rearrange("p (c i j) -> p c i j", i=2, j=NB),
            )

        # h^T = relu(W1^T x^T + b1), stored as fp8 (scaled by S_H).
        # Per-m psum banks with bufs=1: relu(m) drains while the other m's
        # matmuls run, so the next block's matmuls never wait.
        h_sb = actp.tile([128, 2, NB], FP8, tag="h", bufs=5)
        for m in range(2):
            ph = psum.tile([128, NB], F32, tag=f"ph{m}", bufs=1)
            if USE_DR:
                for c in range(4):
                    nc.tensor.matmul(ph[:, :nb], w1_sb[:, c, :, m, :], x_tile[:, c, :, :nb],
                                     perf_mode=DR, start=(c == 0), stop=(c == 3))
            else:
                for c in range(4):
                    for i in range(2):
                        nc.tensor.matmul(ph[:, :nb], w1_sb[:, c, i, m, :], x_tile[:, c, i, :nb],
                                         start=(c == 0 and i == 0), stop=(c == 3 and i == 1))
            if zero_bias:
                nc.vector.tensor_scalar(out=h_sb[:, m, :nb], in0=ph[:, :nb], scalar1=SC_H,
                                        scalar2=0.0, op0=ALU.mult, op1=ALU.max)
            else:
                nc.scalar.activation(out=h_sb[:, m, :nb], in_=ph[:, :nb], func=AF.Relu,
                                     bias=b1_sb[:, m : m + 1], scale=SC_H)
        h_tiles[b] = h_sb

    def gate_a(b):
        """a/t projections + tanh + u = a*t."""
        nb = min(NB, npc - b * NB)
        h_sb = h_tiles[b]

        # a = tanh(h Wa' + ba');  t = tanh(h Wb' + bb')  (0.5s folded host-side)
        # One 4-bank psum tile -> a single merged tanh op over [128, 4, nb].
        pat = psum.tile([128, 4, NB], F32, tag="pat", bufs=1)
        at_sb = actp.tile([128, 4, NB], BF16, tag="at", bufs=4)
        for m in range(2):
            if USE_DR:
                nc.tensor.matmul(pat[:, 0 + m, :nb], wa_sb[:, :, m, :], h_sb[:, :, :nb], perf_mode=DR)
                nc.tensor.matmul(pat[:, 2 + m, :nb], wb_sb[:, :, m, :], h_sb[:, :, :nb], perf_mode=DR)
            else:
                for i in range(2):
                    nc.tensor.matmul(pat[:, 0 + m, :nb], wa_sb[:, i, m, :], h_sb[:, i, :nb],
                                     start=(i == 0), stop=(i == 1))
                for i in range(2):
                    nc.tensor.matmul(pat[:, 2 + m, :nb], wb_sb[:, i, m, :], h_sb[:, i, :nb],
                                     start=(i == 0), stop=(i == 1))
        if zero_bias:
            nc.scalar.activation(out=at_sb[:, :, :nb], in_=pat[:, :, :nb], func=AF.Tanh, scale=SC_AT)
        else:
            for m in range(2):
                nc.scalar.activation(out=at_sb[:, 0 + m, :nb], in_=pat[:, 0 + m, :nb], func=AF.Tanh,
                                     bias=ba_sb[:, m : m + 1], scale=SC_AT)
                nc.scalar.activation(out=at_sb[:, 2 + m, :nb], in_=pat[:, 2 + m, :nb], func=AF.Tanh,
                                     bias=bb_sb[:, m : m + 1], scale=SC_AT)

        # u = a * t  (a*(1+t) = a + a*t is folded into two A-projections)
        u_sb = actp.tile([128, 2, NB], BF16, tag="u", bufs=4)
        nc.vector.tensor_tensor(out=u_sb[:, :, :nb], in0=at_sb[:, 0:2, :nb],
                                in1=at_sb[:, 2:4, :nb], op=ALU.mult)
        g_tiles[b] = (at_sb, u_sb)

    def gate_b(b):
        """A projection, softmax weight, weighted pooling partials."""
        nb = min(NB, npc - b * NB)
        h_sb = h_tiles.pop(b)
        at_sb, u_sb = g_tiles.pop(b)

        # A = (a + a*t) @ (0.5 ac) -> (1, nb);  w = exp(A); Z += sum(w)
        pA = psum.tile([1, NB], F32, tag="pA", bufs=1)
        for k in range(2):
            nc.tensor.matmul(pA[:, :nb], ac_sb[:, k, :], at_sb[:, k, :nb], start=(k == 0), stop=False)
        for k in range(2):
            nc.tensor.matmul(pA[:, :nb], ac_sb[:, k, :], u_sb[:, k, :nb], start=False, stop=(k == 1))
        w_sb = actp.tile([1, NB], BF16, tag="w")
        nc.scalar.activation(out=w_sb[:, :nb], in_=pA[:, :nb], func=AF.Exp, scale=1.0,
                             accum_out=z_parts[:, b : b + 1])

        # broadcast w to all partitions (GpSimd), then S[:,m,b] += rowsum(h/S_H * w)
        wb_bc = actp.tile([128, NB], BF16, tag="wb")
        nc.gpsimd.partition_broadcast(wb_bc[:, :nb], w_sb[:, :nb])
        for m in range(2):
            wf = actp.tile([128, NB], BF16, tag="wf")
            nc.vector.scalar_tensor_tensor(out=wf[:, :nb], in0=h_sb[:, m, :nb], scalar=1.0 / S_H,
                                           in1=wb_bc[:, :nb], op0=ALU.mult, op1=ALU.mult,
                                           accum_out=s_parts[:, m, b : b + 1])

    # Software pipeline: gate_a runs one block late, gate_b three blocks late,
    # so no engine FIFO ever stalls on the cross-engine chain
    # (relu -> a/t MMs -> tanh -> u -> A MMs -> exp -> bcast -> weighted sum)
    # and the PE stays continuously busy (HAM stays warm).
    LAG_B = 3
    for b in range(nblocks):
        h_phase(b)
        if b >= 1:
            gate_a(b - 1)
        if b >= LAG_B:
            gate_b(b - LAG_B)
    gate_a(nblocks - 1)
    for b in range(max(0, nblocks - LAG_B), nblocks):
        gate_b(b)

    nc.sync.dma_start(out=t["s_out"], in_=s_parts)
    nc.sync.dma_start(out=t["z_out"], in_=z_parts)


def build_program(npc: int = NPC, zero_bias: bool = True, enable_asserts: bool = False):
    nblocks = (npc + NB - 1) // NB
    nc = bacc.Bacc("TRN2", target_bir_lowering=False, debug=False, enable_asserts=enable_asserts)

    t = {}
    t["xt"] = nc.dram_tensor("xt", [128, nblocks * 8 * NB], FP8, kind="ExternalInput").ap()
    t["w1q"] = nc.dram_tensor("w1q", [128, 2048], FP8, kind="ExternalInput").ap()
    t["waq"] = nc.dram_tensor("waq", [128, 512], FP8, kind="ExternalInput").ap()
    t["wbq"] = nc.dram_tensor("wbq", [128, 512], FP8, kind="ExternalInput").ap()
    t["ach"] = nc.dram_tensor("ach", [128, 2], BF16, kind="ExternalInput").ap()
    if not zero_bias:
        for nm in ("b1s", "bas", "bbs"):
            t[nm] = nc.dram_tensor(nm, [D_HID], F32, kind="ExternalInput").ap()
    t["s_out"] = nc.dram_tensor("s_out", [128, 2, nblocks], F32, kind="ExternalOutput").ap()
    t["z_out"] = nc.dram_tensor("z_out", [1, nblocks], F32, kind="ExternalOutput").ap()

    with tile.TileContext(nc) as tc, ExitStack() as ctx:
        _build_tile_kernel(ctx, tc, t, npc, nblocks, zero_bias)
    nc.compile()
    return nc


def _q8(a: np.ndarray, scale: float) -> np.ndarray:
    return np.ascontiguousarray((np.asarray(a, np.float32) * scale).astype(NP_FP8))


def make_weight_map(inputs, zero_bias=None):
    W1 = np.asarray(inputs["wsi_w"], np.float64)
    Wv = np.asarray(inputs["wv_w"], np.float64)
    Wa = np.asarray(inputs["aa_w"], np.float64)
    Wb = np.asarray(inputs["ab_w"], np.float64)
    ac = np.asarray(inputs["ac_w"], np.float64)
    bv = np.asarray(inputs["wv_b"], np.float64)
    b1 = np.asarray(inputs["wsi_b"], np.float64)
    ba = np.asarray(bv @ Wa + np.asarray(inputs["aa_b"], np.float64))
    bb = np.asarray(0.5 * (bv @ Wb + np.asarray(inputs["ab_b"], np.float64)))

    # composed gating weights (f folded away); 0.5 of the tanh-sigmoid in Wb'
    Wa_c = Wv @ Wa
    Wb_c = 0.5 * (Wv @ Wb)

    # device layouts
    w1q = _q8(W1, S_W1).reshape(4, 2, 128, 2, 128).transpose(2, 0, 1, 3, 4).reshape(128, 2048)
    waq = _q8(Wa_c, S_WAB).reshape(2, 128, 2, 128).transpose(1, 0, 2, 3).reshape(128, 512)
    wbq = _q8(Wb_c, S_WAB).reshape(2, 128, 2, 128).transpose(1, 0, 2, 3).reshape(128, 512)
    ach = np.ascontiguousarray(
        (0.5 * ac).astype(NP_BF16).reshape(2, 128, 1).transpose(1, 0, 2).reshape(128, 2)
    )
    m = {"w1q": np.ascontiguousarray(w1q), "waq": np.ascontiguousarray(waq),
         "wbq": np.ascontiguousarray(wbq), "ach": ach}
    zb = not (np.any(b1) or np.any(ba) or np.any(bb))
    if not zb:
        m["b1s"] = (np.asarray(b1, np.float32) * S_H).astype(np.float32)
        m["bas"] = np.asarray(ba, np.float32)
        m["bbs"] = np.asarray(bb, np.float32)
    m["_zero_bias"] = zb
    return m


def make_in_maps(x_path, weights, npc: int = NPC, n_cores: int = N_CORES):
    x = np.asarray(x_path[0], np.float32)  # (N, 1024)
    nblocks = (npc + NB - 1) // NB
    npad = nblocks * NB
    w = {k: v for k, v in weights.items() if not k.startswith("_")}
    in_maps = []
    for c in range(n_cores):
        xc = np.zeros((npad, D_IN), np.float32)
        xc[:npc] = x[c * npc : (c + 1) * npc]
        xq = (xc * S_X).astype(NP_FP8)
        # [inst, feat] -> [p, (b c i j)] with feat = c*256 + i*128 + p
        packed = np.ascontiguousarray(
            xq.reshape(nblocks, NB, 4, 2, 128).transpose(4, 0, 2, 3, 1).reshape(128, nblocks * 8 * NB)
        )
        in_maps.append({"xt": packed, **w})
    return in_maps


def finalize(results, c1_w, c1_b, c2_w, c2_b, wv_w, wv_b):
    """Host-side reduction of per-core partials, Wv application + classifier."""
    S = np.zeros((128, 2), np.float64)
    Z = 0.0
    for r in results:
        S += np.asarray(r["s_out"], np.float64).sum(axis=-1)
        Z += float(np.asarray(r["z_out"], np.float64).sum())
    s_vec = S.T.reshape(256)  # feature = m*128 + p
    pooled = (s_vec / Z) @ np.asarray(wv_w, np.float64) + np.asarray(wv_b, np.float64)
    risk = (
        np.maximum(pooled @ np.asarray(c1_w, np.float64) + np.asarray(c1_b, np.float64), 0.0)
        @ np.asarray(c2_w, np.float64)
        + np.asarray(c2_b, np.float64)
    )
    return risk[None, :].astype(np.float32)


_CACHED = {}


def kernel(**inputs) -> np.ndarray:
    weights = make_weight_map(inputs)
    zb = weights["_zero_bias"]
    if zb not in _CACHED:
        _CACHED[zb] = build_program(zero_bias=zb)
    nc = _CACHED[zb]

    in_maps = make_in_maps(np.asarray(inputs["x_path"]), weights)
    res = run_bass_kernel_spmd(nc, in_maps, list(range(N_CORES)))
    return finalize(
        res.results,
        inputs["c1_w"], inputs["c1_b"], inputs["c2_w"], inputs["c2_b"],
        inputs["wv_w"], inputs["wv_b"],
    )
